# revision 17
# baseline (speedup 1.0000x reference)
"""Trainium2 Bass kernel for nn_DecorrLinear.

Data-parallel over batch: core c handles batch element c (B == n_cores == 8).
Each core computes:
  y_c   = x_c @ (W @ Q)^T + bias          (fp32r / TF32 matmuls, fp32 accum)
  sel_c = x_c[sample_idx_c]               (dma_gather)
  sd_c  = sel_c @ Q^T
  m_c   = sd_c^T @ sd_c                   (unscaled partial of the grad GEMM)
  stat_c[p, 0:8]  = per-lane partials of (norm2^2 - s4)
  stat_c[p, 8:16] = per-lane partials of (s4 - 2*norm2 + D)
Host combines: grad = (sum_c m_c) / (2*N) - 0.5*I  (KAPPA=0.5 cancels the
diag terms), losses = partial sums / N / D^2.
"""
import os
from contextlib import ExitStack

import numpy as np

B, L, D, O, NS = 8, 4096, 1024, 2048, 1024
N_CORES = 8
P = 128
KD = D // P     # 8  k-tiles over D
LT = L // P     # 32 l-tiles
NT = NS // P    # 8  sample tiles
OSL = O // 512  # 4  o-slices of 512

_cache = {}


def _build():
    import concourse.bass as bass
    import concourse.tile as tile
    from concourse import bacc, mybir
    from concourse.masks import make_identity

    F32 = mybir.dt.float32
    F32R = mybir.dt.float32r
    I32 = mybir.dt.int32
    I16 = mybir.dt.int16

    skip_stats = bool(os.environ.get("K_SKIP_STATS"))
    skip_g = bool(os.environ.get("K_SKIP_G"))
    skip_main = bool(os.environ.get("K_SKIP_MAIN")) or skip_g
    LVL = int(os.environ.get("K_STATS_LEVEL", "4"))

    nc = bacc.Bacc("TRN2", target_bir_lowering=False, debug=False,
                   num_devices=N_CORES)
    x_d = nc.dram_tensor("x", [L, D], F32, kind="ExternalInput").ap()
    w_d = nc.dram_tensor("w", [O, D], F32, kind="ExternalInput").ap()
    dec_d = nc.dram_tensor("dec", [D, D], F32, kind="ExternalInput").ap()
    bias_d = nc.dram_tensor("bias", [O], F32, kind="ExternalInput").ap()
    sidx_d = nc.dram_tensor("sidx", [NS], I32, kind="ExternalInput").ap()
    y_d = nc.dram_tensor("y", [L, O], F32, kind="ExternalOutput").ap()
    m_d = nc.dram_tensor("m", [D, D], F32, kind="ExternalOutput").ap()
    stat_d = nc.dram_tensor("stat", [P, 2 * NT], F32, kind="ExternalOutput").ap()

    ADD = mybir.AluOpType.add
    MULT = mybir.AluOpType.mult
    SUB = mybir.AluOpType.subtract

    with tile.TileContext(nc) as tc, ExitStack() as ctx:
        cpool = ctx.enter_context(tc.tile_pool(name="const", bufs=1))
        psA = ctx.enter_context(tc.tile_pool(name="psA", bufs=2, space="PSUM"))
        psB = ctx.enter_context(tc.tile_pool(name="psB", bufs=2, space="PSUM"))
        psAcc = ctx.enter_context(tc.tile_pool(name="psAcc", bufs=2, space="PSUM"))

        # ---- constants ----
        ident_f = cpool.tile([P, P], F32)
        make_identity(nc, ident_f[:])
        ident_r = cpool.tile([P, P], F32R)
        nc.vector.tensor_copy(ident_r[:], ident_f[:])

        bias_bc = cpool.tile([P, O], F32)
        bias_bcast_ap = bass.AP(tensor=bias_d.tensor, offset=bias_d.offset,
                                ap=[[0, P]] + list(bias_d.ap))
        nc.sync.dma_start(out=bias_bc[:], in_=bias_bcast_ap)

        # dma_gather ucode: [16, NS//16] int16 index block, replicated into
        # each Q7 core's 16-partition group (cast int32->int16 in the DMA).
        sidx16 = cpool.tile([P, NS // 16], I16)
        for g in range(8):
            nc.gpsimd.dma_start(
                out=sidx16[16 * g:16 * (g + 1), :],
                in_=sidx_d.rearrange("(c p) -> p c", p=16))

        stat_sb = cpool.tile([P, 2 * NT], F32)

        # ================= stats phase =================
        with tc.tile_pool(name="sd", bufs=1) as sdp, \
             tc.tile_pool(name="stt", bufs=2) as stp:
            with tc.tile_pool(name="decT", bufs=1) as decTp, \
                 tc.tile_pool(name="sel", bufs=1) as selp, \
                 tc.tile_pool(name="selT", bufs=2) as selTp:
                # decT[k][d-in-block, e] = decorr[e, d]^T tiles (f32r)
                decT = [decTp.tile([P, D], F32R, tag=f"dT{k}", name=f"dT{k}")
                        for k in range(KD)]
                with tc.tile_pool(name="decnat1", bufs=2) as dp1:
                    for e in range(KD):
                        dn = dp1.tile([P, D], F32R, tag="dn", name=f"dna{e}")
                        nc.gpsimd.dma_start(out=dn[:],
                                            in_=dec_d[e * P:(e + 1) * P, :])
                        for k in range(KD):
                            pst = psA.tile([P, P], F32R)
                            nc.tensor.transpose(
                                pst[:], dn[:, k * P:(k + 1) * P], ident_r[:])
                            nc.vector.tensor_copy(
                                decT[k][:, e * P:(e + 1) * P], pst[:])

                sd_t = []
                if not skip_stats:
                    selall = selp.tile([P, NT, D], F32)
                    selall_r = selp.tile([P, NT, D], F32R, tag="selr",
                                         name="selall_r")
                    gather_sem = nc.alloc_semaphore("gather_sem")
                    with tc.tile_critical():
                        # Tile does not model InstDMAGatherAnt as a DMA, so
                        # gate consumers on the SDMA completion sem manually
                        # via this cast-copy (f32 -> f32r).
                        nc.gpsimd.dma_gather(
                            out_ap=selall[:], in_ap=x_d, idxs_ap=sidx16[:],
                            num_idxs=NS, num_idxs_reg=NS,
                            elem_size=D).then_inc(gather_sem, 16)
                        nc.vector.tensor_copy(
                            selall_r[:], selall[:])._wait_ge(gather_sem, 16)
                for ti in range(0 if skip_stats else NT):
                    sel = selall_r[:, ti, :]
                    if LVL <= 1:
                        nc.sync.dma_start(
                            out=m_d[ti * P:(ti + 1) * P, :],
                            in_=sel[:].bitcast(F32))
                        continue
                    selT = selTp.tile([P, D], F32R)
                    for k in range(KD):
                        pst = psB.tile([P, P], F32R)
                        nc.tensor.transpose(
                            pst[:], sel[:, k * P:(k + 1) * P], ident_r[:])
                        nc.vector.tensor_copy(
                            selT[:, k * P:(k + 1) * P], pst[:])
                    acc = psAcc.tile([P, D], F32, tag="acc")
                    for k in range(KD):
                        for s in range(2):
                            nc.tensor.matmul(
                                acc[:, s * 512:(s + 1) * 512],
                                selT[:, k * P:(k + 1) * P],
                                decT[k][:, s * 512:(s + 1) * 512],
                                start=(k == 0), stop=(k == KD - 1))
                    sdt = sdp.tile([P, D], F32R, tag=f"sd{ti}", name=f"sd{ti}")
                    nc.vector.tensor_copy(sdt[:], acc[:])
                    if LVL <= 2:
                        nc.sync.dma_start(
                            out=m_d[ti * P:(ti + 1) * P, :],
                            in_=sdt[:].bitcast(F32))
                        continue
                    # per-sample stats from the (tf32-rounded) SBUF copy
                    # tensor_tensor_reduce is broken on HW; use ACT Square
                    # + DVE reduce_sum instead.
                    sdf = sdt[:].bitcast(F32)
                    SQ = mybir.ActivationFunctionType.Square
                    s2 = stp.tile([P, D], F32, tag="s2")
                    n2 = stp.tile([P, 1], F32, tag="n2")
                    nc.scalar.activation(out=s2[:], in_=sdf, func=SQ)
                    nc.vector.reduce_sum(out=n2[:], in_=s2[:],
                                         axis=mybir.AxisListType.X)
                    s2sq = stp.tile([P, D], F32, tag="s2sq")
                    s4 = stp.tile([P, 1], F32, tag="s4")
                    nc.scalar.activation(out=s2sq[:], in_=s2[:], func=SQ)
                    nc.vector.reduce_sum(out=s4[:], in_=s2sq[:],
                                         axis=mybir.AxisListType.X)
                    n2sq = stp.tile([P, 1], F32, tag="n2sq")
                    nc.vector.tensor_tensor(
                        out=n2sq[:], in0=n2[:], in1=n2[:], op=MULT)
                    nc.vector.tensor_tensor(
                        out=stat_sb[:, ti:ti + 1], in0=n2sq[:], in1=s4[:],
                        op=SUB)
                    t1 = stp.tile([P, 1], F32, tag="t1")
                    nc.vector.tensor_scalar(
                        out=t1[:], in0=n2[:], scalar1=-2.0,
                        scalar2=float(D), op0=MULT, op1=ADD)
                    nc.vector.tensor_tensor(
                        out=stat_sb[:, NT + ti:NT + ti + 1], in0=t1[:],
                        in1=s4[:], op=ADD)
                    sd_t.append(sdt)

            if not skip_stats and LVL >= 3:
                nc.sync.dma_start(out=stat_d, in_=stat_sb[:])

            # ---- m = sd^T @ sd (unscaled) ----
            with tc.tile_pool(name="mout", bufs=2) as moutp:
                n_m = KD if (not skip_stats and LVL >= 4) else 0
                for i in range(n_m):
                    acc = psAcc.tile([P, D], F32, tag="acc")
                    for ti in range(NT):
                        for s in range(2):
                            nc.tensor.matmul(
                                acc[:, s * 512:(s + 1) * 512],
                                sd_t[ti][:, i * P:(i + 1) * P],
                                sd_t[ti][:, s * 512:(s + 1) * 512],
                                start=(ti == 0), stop=(ti == NT - 1))
                    mo = moutp.tile([P, D], F32, tag="mo")
                    nc.vector.tensor_copy(mo[:], acc[:])
                    nc.sync.dma_start(out=m_d[i * P:(i + 1) * P, :],
                                      in_=mo[:])

        # ================= G build + main GEMM =================
        with tc.tile_pool(name="g", bufs=1) as gpool:
            g_t = [gpool.tile([P, O], F32R, tag=f"g{i}", name=f"g{i}")
                   for i in range(KD)]
            with tc.tile_pool(name="decnat2", bufs=1) as dp2, \
                 tc.tile_pool(name="win", bufs=2) as winp, \
                 tc.tile_pool(name="wt", bufs=2) as wtp:
                dec_nat = []
                for e in range(0 if skip_g else KD):
                    dn = dp2.tile([P, D], F32R, tag=f"dn{e}", name=f"dnb{e}")
                    nc.gpsimd.dma_start(out=dn[:],
                                        in_=dec_d[e * P:(e + 1) * P, :])
                    dec_nat.append(dn)
                for j in range(0 if skip_g else OSL):
                    wt = [wtp.tile([P, 512], F32R, tag=f"wt{e}", name=f"wt{e}")
                          for e in range(KD)]
                    for ob in range(4):
                        win = winp.tile([P, D], F32R, tag="win")
                        r0 = (j * 4 + ob) * P
                        nc.gpsimd.dma_start(out=win[:], in_=w_d[r0:r0 + P, :])
                        for e in range(KD):
                            pst = psA.tile([P, P], F32R)
                            nc.tensor.transpose(
                                pst[:], win[:, e * P:(e + 1) * P], ident_r[:])
                            nc.vector.tensor_copy(
                                wt[e][:, ob * P:(ob + 1) * P], pst[:])
                    for i in range(KD):
                        acc = psAcc.tile([P, D], F32, tag="acc")
                        for e in range(KD):
                            nc.tensor.matmul(
                                acc[:, 0:512],
                                dec_nat[e][:, i * P:(i + 1) * P],
                                wt[e][:],
                                start=(e == 0), stop=(e == KD - 1))
                        nc.vector.tensor_copy(
                            g_t[i][:, j * 512:(j + 1) * 512], acc[:, 0:512])

            # ---- main GEMM: y = x @ G + bias ----
            with tc.tile_pool(name="xin", bufs=3) as xinp, \
                 tc.tile_pool(name="xt", bufs=2) as xtp, \
                 tc.tile_pool(name="yout", bufs=2) as youtp:
                for li in range(0 if skip_main else LT):
                    xin = xinp.tile([P, D], F32R, tag="xin")
                    nc.gpsimd.dma_start(out=xin[:],
                                        in_=x_d[li * P:(li + 1) * P, :])
                    xt = xtp.tile([P, D], F32R, tag="xt")
                    for k in range(KD):
                        pst = psA.tile([P, P], F32R)
                        nc.tensor.transpose(
                            pst[:], xin[:, k * P:(k + 1) * P], ident_r[:])
                        nc.vector.tensor_copy(xt[:, k * P:(k + 1) * P], pst[:])
                    yo = youtp.tile([P, O], F32, tag="yo")
                    for h in range(2):
                        acc = psAcc.tile([P, D], F32, tag="acc")
                        for k in range(KD):
                            for s in range(2):
                                o0 = h * 1024 + s * 512
                                nc.tensor.matmul(
                                    acc[:, s * 512:(s + 1) * 512],
                                    xt[:, k * P:(k + 1) * P],
                                    g_t[k][:, o0:o0 + 512],
                                    start=(k == 0), stop=(k == KD - 1))
                        nc.vector.tensor_tensor(
                            out=yo[:, h * 1024:(h + 1) * 1024], in0=acc[:],
                            in1=bias_bc[:, h * 1024:(h + 1) * 1024], op=ADD)
                    nc.sync.dma_start(out=y_d[li * P:(li + 1) * P, :],
                                      in_=yo[:])

    nc.compile()
    return nc


def _get_nc():
    if "nc" not in _cache:
        _cache["nc"] = _build()
    return _cache["nc"]


def kernel(x, weight, bias, decorr, sample_idx):
    from concourse.bass_utils import run_bass_kernel_spmd

    x = np.ascontiguousarray(np.asarray(x, dtype=np.float32))
    weight = np.ascontiguousarray(np.asarray(weight, dtype=np.float32))
    bias = np.ascontiguousarray(np.asarray(bias, dtype=np.float32))
    decorr = np.ascontiguousarray(np.asarray(decorr, dtype=np.float32))
    sidx = np.ascontiguousarray(np.asarray(sample_idx).astype(np.int32))

    nc = _get_nc()
    in_maps = [{"x": x[c], "w": weight, "dec": decorr, "bias": bias,
                "sidx": sidx[c]} for c in range(N_CORES)]
    res = run_bass_kernel_spmd(nc, in_maps, list(range(N_CORES))).results

    y = np.stack([res[c]["y"] for c in range(N_CORES)], axis=0)

    n_total = B * NS
    m_sum = np.zeros((D, D), dtype=np.float64)
    corr_sum = 0.0
    whit_sum = 0.0
    for c in range(N_CORES):
        m_sum += res[c]["m"].astype(np.float64)
        st = res[c]["stat"].astype(np.float64)
        corr_sum += st[:, :NT].sum()
        whit_sum += st[:, NT:].sum()

    grad = (m_sum / (2.0 * n_total) - 0.5 * np.eye(D)).astype(np.float32)
    inv_dd = 1.0 / (D * D)
    corr = np.float32(corr_sum / n_total * inv_dd)
    whit = np.float32(whit_sum / n_total * inv_dd)
    return y, grad, corr, whit
